# revision 3
# baseline (speedup 1.0000x reference)
"""MultiHeadAttention Trainium2 kernel (8-core batch-parallel).

Reference computation (per batch b):
    K = k @ Wk + bk ; V = v @ Wv + bv ; Q = (q @ Wq + bq) * (1/8)
    per head h: scores = Qh @ Kh^T ; scores[mask!=0] = -inf
    attn = softmax(scores, axis=-1)
    context_h = attn @ Vh ; output = concat(context) @ Wo + bo
    attn_mean = sum_h(attn) / 16

Sharding: pure data-parallel over batch (B=8 -> one batch per core).

Per-core design ("transposed softmax"):
  - All matmul inputs in bf16, accumulation fp32 in PSUM.
  - Projections produce Q^T, K^T ([d_head*H, S] layout) and V_ext
    (natural [S, H, 65] layout, column 64 = ones for fused row sums).
  - scoresT[s_k, s_q] computed directly by PE (contraction dim = 64).
  - exp on ScalarE evacuates PSUM -> SBUF bf16; mask applied as a
    multiply by keepT = (maskT == 0) (exp(s + (-inf)*m) == exp(s)*keep).
  - PV matmul: ctxU^T[65, s_q] = V_ext^T @ expTm accumulated over s_k
    tiles; row 64 gives softmax denominators (sums over s_k).
  - r = 1/sums broadcast to [128, S] via a K=1 PE matmul; context and
    attn_mean are normalized with tensor_tensor multiplies.
  - attn_mean accumulated transposed in bf16, PE-transposed at the end.
  - output = (ctxT)^T @ Wo via PE with ctxT as stationary (natural out).
"""

import numpy as np

import concourse.bass as bass
import concourse.mybir as mybir
import concourse.tile as tile
from concourse import bacc
from concourse.masks import make_identity

F32 = mybir.dt.float32
BF16 = mybir.dt.bfloat16
I32 = mybir.dt.int32
AF = mybir.ActivationFunctionType
OP = mybir.AluOpType

B = 8
S = 1024
D = 1024
H = 16
DH = 64
P = 128


def build_attention_nc(s=S, h=H, debug=False):
    """Build the per-core Bass program. s = sequence length, h = heads."""
    d = D  # model dim fixed by the weight shapes
    nt = d // P          # tiles along d (8)
    st = s // P          # tiles along s
    nb = max(1, s // 512)  # 512-wide column blocks per s
    bw = s // nb         # block width (512 normally)
    hpt = P // DH        # heads per 128-partition tile (2)

    nc = bacc.Bacc("TRN2", target_bir_lowering=False, debug=debug)

    dq = nc.dram_tensor("q", [s, d], F32, kind="ExternalInput")
    dk = nc.dram_tensor("k", [s, d], F32, kind="ExternalInput")
    dv = nc.dram_tensor("v", [s, d], F32, kind="ExternalInput")
    dmask = nc.dram_tensor("attn_mask", [s, s], I32, kind="ExternalInput")
    dWq = nc.dram_tensor("Wq", [d, d], F32, kind="ExternalInput")
    dWk = nc.dram_tensor("Wk", [d, d], F32, kind="ExternalInput")
    dWv = nc.dram_tensor("Wv", [d, d], F32, kind="ExternalInput")
    dWo = nc.dram_tensor("Wo", [d, d], F32, kind="ExternalInput")
    dbq = nc.dram_tensor("bq", [d], F32, kind="ExternalInput")
    dbk = nc.dram_tensor("bk", [d], F32, kind="ExternalInput")
    dbv = nc.dram_tensor("bv", [d], F32, kind="ExternalInput")
    dbo = nc.dram_tensor("bo", [d], F32, kind="ExternalInput")
    dout = nc.dram_tensor("output", [s, d], F32, kind="ExternalOutput")
    dmean = nc.dram_tensor("attn_mean", [s, s], F32, kind="ExternalOutput")

    with tile.TileContext(nc) as tc:
        with (
            tc.tile_pool(name="persist", bufs=1) as persist,
            tc.tile_pool(name="consts", bufs=1) as consts,
            tc.tile_pool(name="dram", bufs=1, space="DRAM") as dram,
        ):
            # ---------- constants ----------
            ident = consts.tile([P, P], BF16)
            make_identity(nc, ident)
            ones_row = consts.tile([1, max(s, P)], BF16)
            nc.vector.memset(ones_row, 1.0)

            # biases as bf16 rows [1, d]
            brows = {}
            for nm, dt_ in (("bq", dbq), ("bk", dbk), ("bv", dbv), ("bo", dbo)):
                rf = consts.tile([1, d], F32, tag=f"{nm}f")
                nc.sync.dma_start(out=rf, in_=dt_[None, :])
                rb = consts.tile([1, d], BF16, tag=f"{nm}b")
                nc.vector.tensor_copy(out=rb, in_=rf)
                brows[nm] = rb

            # persistent big tensors
            QT = persist.tile([P, nt, s], BF16)    # [d, s] transposed Q (prescaled)
            KT = persist.tile([P, nt, s], BF16)
            Vx = persist.tile([P, st, h, DH + 1], BF16)  # V_ext natural + ones col
            keepT = persist.tile([P, st, s], BF16)  # (mask^T == 0) as bf16
            ctxT = persist.tile([P, nt, s], BF16)   # context transposed
            meanT = persist.tile([P, st, s], BF16)  # sum_h attn^T (bf16 accum)
            nc.vector.memset(meanT, 0.0)

            # ---------- phase 0: transpose inputs & make keepT ----------
            with (
                tc.tile_pool(name="xT", bufs=2) as xTp,
                tc.tile_pool(name="stage", bufs=2) as stage,
                tc.tile_pool(name="wpool", bufs=1) as wpool,
                tc.tile_pool(name="ppj", bufs=2, space="PSUM") as ppj,
            ):
                def transposed_input(nm, src):
                    """DRAM f32 [s,d] -> SBUF bf16 [d,s] via cast + xbar bounce."""
                    scratch = dram.tile([s, d], BF16, tag=f"sc_{nm}")
                    for i in range(st):
                        t = stage.tile([P, d], BF16, tag="stage_in")
                        nc.gpsimd.dma_start(out=t, in_=src[i * P:(i + 1) * P, :])
                        nc.sync.dma_start(out=scratch[i * P:(i + 1) * P, :], in_=t)
                    xT = xTp.tile([P, nt, s], BF16, tag="xT")
                    for j in range(nt):
                        nc.sync.dma_start_transpose(
                            out=xT[:, j, :], in_=scratch[:, j * P:(j + 1) * P]
                        )
                    return xT

                # keepT = (mask^T == 0)
                kscratch = dram.tile([s, s], BF16, tag="sc_keep")
                for i in range(s // P):
                    mi = stage.tile([P, s], I32, tag="mask_i32")
                    nc.sync.dma_start(out=mi, in_=dmask[i * P:(i + 1) * P, :])
                    mb = stage.tile([P, s], BF16, tag="mask_bf")
                    nc.vector.tensor_scalar(
                        out=mb, in0=mi, scalar1=0, scalar2=None, op0=OP.is_equal
                    )
                    nc.sync.dma_start(out=kscratch[i * P:(i + 1) * P, :], in_=mb)
                for j in range(st):
                    nc.sync.dma_start_transpose(
                        out=keepT[:, j, :], in_=kscratch[:, j * P:(j + 1) * P]
                    )

                # ---------- phase 0b: projections ----------
                def proj_T(w_dram, x_nm, x_dram, outbuf, bias_row, scale):
                    """outbuf[dout, s] (transposed): lhsT=W[din,dout], rhs=xT."""
                    x_T = transposed_input(x_nm, x_dram)
                    wsb = wpool.tile([P, nt, d], BF16, tag="w")
                    for kt in range(nt):
                        nc.gpsimd.dma_start(
                            out=wsb[:, kt, :], in_=w_dram[kt * P:(kt + 1) * P, :]
                        )
                    for mt in range(nt):
                        ps = ppj.tile([P, s], F32, tag="pj")
                        for n2 in range(nb):
                            cb = slice(n2 * bw, (n2 + 1) * bw)
                            for kt in range(nt):
                                nc.tensor.matmul(
                                    ps[:, cb],
                                    lhsT=wsb[:, kt, mt * P:(mt + 1) * P],
                                    rhs=x_T[:, kt, cb],
                                    start=(kt == 0),
                                    stop=False,
                                )
                            # bias: out[m, n] += bias[m] * 1
                            nc.tensor.matmul(
                                ps[:, cb],
                                lhsT=bias_row[0:1, mt * P:(mt + 1) * P],
                                rhs=ones_row[0:1, 0:bw],
                                start=False,
                                stop=True,
                            )
                        nc.scalar.activation(
                            out=outbuf[:, mt, :], in_=ps, func=AF.Copy, scale=scale
                        )

                proj_T(dWq, "q", dq, QT, brows["bq"], 1.0 / 8.0)
                proj_T(dWk, "k", dk, KT, brows["bk"], 1.0)

                # V natural: out[s, dout]: lhsT = vT[din, s-tile], rhs = Wv
                vT = transposed_input("v", dv)
                wsb = wpool.tile([P, nt, d], BF16, tag="w")
                for kt in range(nt):
                    nc.gpsimd.dma_start(
                        out=wsb[:, kt, :], in_=dWv[kt * P:(kt + 1) * P, :]
                    )
                for mt in range(st):
                    ps = ppj.tile([P, d], F32, tag="pj")
                    for n2 in range(d // bw if d >= bw else 1):
                        cbw = min(bw, d)
                        cb = slice(n2 * cbw, (n2 + 1) * cbw)
                        for kt in range(nt):
                            nc.tensor.matmul(
                                ps[:, cb],
                                lhsT=vT[:, kt, mt * P:(mt + 1) * P],
                                rhs=wsb[:, kt, cb],
                                start=(kt == 0),
                                stop=False,
                            )
                        nc.tensor.matmul(
                            ps[:, cb],
                            lhsT=ones_row[0:1, 0:P],
                            rhs=brows["bv"][0:1, cb],
                            start=False,
                            stop=True,
                        )
                    nc.scalar.activation(
                        out=Vx[:, mt, :, 0:DH],
                        in_=ps.rearrange("p (h dh) -> p h dh", h=h),
                        func=AF.Copy,
                    )
                nc.vector.memset(Vx[:, :, :, DH:DH + 1], 1.0)

            # ---------- heads ----------
            with (
                tc.tile_pool(name="psc", bufs=2, space="PSUM") as psc_pool,
                tc.tile_pool(name="ppv", bufs=1, space="PSUM") as ppv_pool,
                tc.tile_pool(name="prb", bufs=1, space="PSUM") as prb_pool,
                tc.tile_pool(name="hwork", bufs=2) as hwork,
                tc.tile_pool(name="expp", bufs=1) as expp,
            ):
                for hh in range(h):
                    ht, ho = hh // hpt, (hh % hpt) * DH
                    expT = expp.tile([P, st, s], BF16, tag="expT")
                    ppv = ppv_pool.tile([DH + 1, s], F32, tag="pv")
                    for kt in range(st):
                        pss = psc_pool.tile([P, s], F32, tag="sc")
                        for n2 in range(nb):
                            cb = slice(n2 * bw, (n2 + 1) * bw)
                            nc.tensor.matmul(
                                pss[:, cb],
                                lhsT=KT[ho:ho + DH, ht, kt * P:(kt + 1) * P],
                                rhs=QT[ho:ho + DH, ht, cb],
                                start=True,
                                stop=True,
                            )
                        nc.scalar.activation(out=expT[:, kt, :], in_=pss, func=AF.Exp)
                        nc.vector.tensor_tensor(
                            out=expT[:, kt, :], in0=expT[:, kt, :],
                            in1=keepT[:, kt, :], op=OP.mult,
                        )
                        for n2 in range(nb):
                            cb = slice(n2 * bw, (n2 + 1) * bw)
                            nc.tensor.matmul(
                                ppv[:, cb],
                                lhsT=Vx[:, kt, hh, :],
                                rhs=expT[:, kt, cb],
                                start=(kt == 0),
                                stop=(kt == st - 1),
                            )
                    # softmax denominators -> r (bf16 row)
                    rrow = hwork.tile([1, s], F32, tag="rrow")
                    nc.vector.reciprocal(out=rrow, in_=ppv[DH:DH + 1, :])
                    rrow_b = hwork.tile([1, s], BF16, tag="rrowb")
                    nc.vector.tensor_copy(out=rrow_b, in_=rrow)
                    prb = prb_pool.tile([P, s], F32, tag="rb")
                    for n2 in range(nb):
                        cb = slice(n2 * bw, (n2 + 1) * bw)
                        nc.tensor.matmul(
                            prb[:, cb], lhsT=ones_row[0:1, 0:P],
                            rhs=rrow_b[0:1, cb], start=True, stop=True,
                        )
                    rb = hwork.tile([P, s], BF16, tag="rb_sb")
                    nc.scalar.activation(out=rb, in_=prb, func=AF.Copy)
                    # context normalize (psum fp32 * rb -> bf16 sbuf)
                    nc.vector.tensor_tensor(
                        out=ctxT[ho:ho + DH, ht, :], in0=ppv[0:DH, :],
                        in1=rb[0:DH, :], op=OP.mult,
                    )
                    # attn_mean accumulation (transposed, bf16)
                    for kt in range(st):
                        at = hwork.tile([P, s], BF16, tag="attn_tmp")
                        nc.vector.tensor_tensor(
                            out=at, in0=expT[:, kt, :], in1=rb, op=OP.mult
                        )
                        nc.vector.tensor_tensor(
                            out=meanT[:, kt, :], in0=at, in1=meanT[:, kt, :],
                            op=OP.add,
                        )

            # ---------- epilogue: attn_mean transpose + output proj ----------
            with (
                tc.tile_pool(name="pep", bufs=2, space="PSUM") as pep,
                tc.tile_pool(name="osb", bufs=2) as osb,
                tc.tile_pool(name="wo", bufs=1) as wop,
            ):
                for mt in range(st):
                    psm = pep.tile([P, s], BF16, tag="mt")
                    for kt in range(st):
                        nc.tensor.transpose(
                            out=psm[:, kt * P:(kt + 1) * P],
                            in_=meanT[:, kt, mt * P:(mt + 1) * P],
                            identity=ident,
                        )
                    mo = osb.tile([P, s], F32, tag="mean_out")
                    nc.scalar.activation(
                        out=mo, in_=psm, func=AF.Copy, scale=1.0 / h
                    )
                    nc.sync.dma_start(out=dmean[mt * P:(mt + 1) * P, :], in_=mo)

                wsb = wop.tile([P, nt, d], BF16)
                for kt in range(nt):
                    nc.gpsimd.dma_start(
                        out=wsb[:, kt, :], in_=dWo[kt * P:(kt + 1) * P, :]
                    )
                for mt in range(st):
                    pso = pep.tile([P, d], F32, tag="out")
                    for n2 in range(d // bw if d >= bw else 1):
                        cbw = min(bw, d)
                        cb = slice(n2 * cbw, (n2 + 1) * cbw)
                        for kt in range(nt):
                            nc.tensor.matmul(
                                pso[:, cb],
                                lhsT=ctxT[:, kt, mt * P:(mt + 1) * P],
                                rhs=wsb[:, kt, cb],
                                start=(kt == 0),
                                stop=False,
                            )
                        nc.tensor.matmul(
                            pso[:, cb],
                            lhsT=ones_row[0:1, 0:P],
                            rhs=brows["bo"][0:1, cb],
                            start=False,
                            stop=True,
                        )
                    oo = osb.tile([P, d], F32, tag="out_sb")
                    nc.scalar.activation(out=oo, in_=pso, func=AF.Copy)
                    nc.sync.dma_start(out=dout[mt * P:(mt + 1) * P, :], in_=oo)

    nc.compile()
    return nc


_NC_CACHE = {}


def _get_nc():
    if "nc" not in _NC_CACHE:
        _NC_CACHE["nc"] = build_attention_nc()
    return _NC_CACHE["nc"]


def kernel(k, v, q, attn_mask, Wk, bk, Wv, bv, Wq, bq, Wo, bo, **_ignored):
    from concourse.bass_utils import run_bass_kernel_spmd

    k = np.asarray(k, np.float32)
    v = np.asarray(v, np.float32)
    q = np.asarray(q, np.float32)
    attn_mask = np.asarray(attn_mask, np.int32)
    shared = {
        "Wk": np.asarray(Wk, np.float32), "bk": np.asarray(bk, np.float32),
        "Wv": np.asarray(Wv, np.float32), "bv": np.asarray(bv, np.float32),
        "Wq": np.asarray(Wq, np.float32), "bq": np.asarray(bq, np.float32),
        "Wo": np.asarray(Wo, np.float32), "bo": np.asarray(bo, np.float32),
    }
    in_maps = []
    for b in range(B):
        m = {"q": q[b], "k": k[b], "v": v[b], "attn_mask": attn_mask[b]}
        m.update(shared)
        in_maps.append(m)

    nc = _get_nc()
    res = run_bass_kernel_spmd(nc, in_maps, core_ids=list(range(B)))
    output = np.stack([res.results[b]["output"] for b in range(B)])
    attn_mean = np.stack([res.results[b]["attn_mean"] for b in range(B)])
    return output, attn_mean


# revision 12
# speedup vs baseline: 1.4297x; 1.4297x over previous
"""MultiHeadAttention Trainium2 kernel (8-core batch-parallel).

Reference computation (per batch b):
    K = k @ Wk + bk ; V = v @ Wv + bv ; Q = (q @ Wq + bq) * (1/8)
    per head h: scores = Qh @ Kh^T ; scores[mask!=0] = -inf
    attn = softmax(scores, axis=-1)
    context_h = attn @ Vh ; output = concat(context) @ Wo + bo
    attn_mean = sum_h(attn) / 16

Sharding: pure data-parallel over batch (B=8 -> one batch per core).

Per-core design ("transposed softmax"):
  - All matmul inputs in bf16, accumulation fp32 in PSUM.
  - Projections produce Q^T, K^T ([d_head*H, S] layout) and V_ext
    (natural [S, H, 65] layout, column 64 = ones for fused row sums).
  - scoresT[s_k, s_q] computed directly by PE (contraction dim = 64).
  - exp on ScalarE evacuates PSUM -> SBUF bf16; mask applied as a
    multiply by keepT = (maskT == 0) (exp(s + (-inf)*m) == exp(s)*keep).
  - PV matmul: ctxU^T[65, s_q] = V_ext^T @ expTm accumulated over s_k
    tiles; row 64 gives softmax denominators (sums over s_k).
  - r = 1/sums broadcast to [128, S] via a K=1 PE matmul; context and
    attn_mean are normalized with tensor_tensor multiplies.
  - attn_mean accumulated transposed in bf16, PE-transposed at the end.
  - output = (ctxT)^T @ Wo via PE with ctxT as stationary (natural out).
"""

import numpy as np

import concourse.bass as bass
import concourse.mybir as mybir
import concourse.tile as tile
from concourse import bacc
from concourse.masks import make_identity

F32 = mybir.dt.float32
BF16 = mybir.dt.bfloat16
I32 = mybir.dt.int32
AF = mybir.ActivationFunctionType
OP = mybir.AluOpType

B = 8
S = 1024
D = 1024
H = 16
DH = 64
P = 128


def build_attention_nc(s=S, h=H, debug=False):
    """Build the per-core Bass program. s = sequence length, h = heads."""
    d = D  # model dim fixed by the weight shapes
    nt = d // P          # tiles along d (8)
    st = s // P          # tiles along s
    nb = max(1, s // 512)  # 512-wide column blocks per s
    bw = s // nb         # block width (512 normally)
    hpt = P // DH        # heads per 128-partition tile (2)

    nc = bacc.Bacc("TRN2", target_bir_lowering=False, debug=debug)

    dq = nc.dram_tensor("q", [s, d], F32, kind="ExternalInput")
    dk = nc.dram_tensor("k", [s, d], F32, kind="ExternalInput")
    dv = nc.dram_tensor("v", [s, d], F32, kind="ExternalInput")
    dmask = nc.dram_tensor("attn_mask", [s, s], I32, kind="ExternalInput")
    dWq = nc.dram_tensor("Wq", [d, d], F32, kind="ExternalInput")
    dWk = nc.dram_tensor("Wk", [d, d], F32, kind="ExternalInput")
    dWv = nc.dram_tensor("Wv", [d, d], F32, kind="ExternalInput")
    dWo = nc.dram_tensor("Wo", [d, d], F32, kind="ExternalInput")
    dbq = nc.dram_tensor("bq", [d], F32, kind="ExternalInput")
    dbk = nc.dram_tensor("bk", [d], F32, kind="ExternalInput")
    dbv = nc.dram_tensor("bv", [d], F32, kind="ExternalInput")
    dbo = nc.dram_tensor("bo", [d], F32, kind="ExternalInput")
    dout = nc.dram_tensor("output", [s, d], F32, kind="ExternalOutput")
    dmean = nc.dram_tensor("attn_mean", [s, s], F32, kind="ExternalOutput")

    with tile.TileContext(nc) as tc:
        with (
            tc.tile_pool(name="persist", bufs=1) as persist,
            tc.tile_pool(name="consts", bufs=1) as consts,
            tc.tile_pool(name="dram", bufs=1, space="DRAM") as dram,
        ):
            # ---------- constants ----------
            ident = consts.tile([P, P], BF16)
            make_identity(nc, ident)
            ones_row = consts.tile([1, max(s, P)], BF16)
            nc.vector.memset(ones_row, 1.0)
            ones_f32 = consts.tile([1, 1], F32)
            nc.vector.memset(ones_f32, 1.0)
            # onehot[i, j*P + c] = (i == j): stationary for row-broadcasts
            onehot = consts.tile([st, st, P], BF16)
            nc.gpsimd.memset(onehot, 0.0)
            nc.gpsimd.affine_select(
                out=onehot, in_=onehot, compare_op=OP.not_equal, fill=1.0,
                base=0, pattern=[[-1, st], [0, P]], channel_multiplier=1,
            )

            # biases as bf16 rows [1, d]
            brows = {}
            for nm, dt_ in (("bq", dbq), ("bk", dbk), ("bv", dbv), ("bo", dbo)):
                rf = consts.tile([1, d], F32, tag=f"{nm}f")
                nc.sync.dma_start(out=rf, in_=dt_[None, :])
                rb = consts.tile([1, d], BF16, tag=f"{nm}b")
                nc.vector.tensor_copy(out=rb, in_=rf)
                brows[nm] = rb

            # persistent big tensors
            QT = persist.tile([P, nt, s], BF16)    # [d, s] transposed Q (prescaled)
            KT = persist.tile([P, nt, s], BF16)
            Vx = persist.tile([P, st, h, DH + 1], BF16)  # V_ext natural + ones col
            keepT = persist.tile([P, st, s], BF16)  # (mask^T == 0) as bf16
            ctxT = persist.tile([P, nt, s], BF16)   # context transposed

            # attn_mean accumulator lives in DRAM (SWDGE accumulate-DMA)
            meanT_dram = dram.tile([s, s], BF16, tag="meanT")

            # ---------- phase 0: transpose inputs & make keepT ----------
            with (
                tc.tile_pool(name="xT", bufs=2) as xTp,
                tc.tile_pool(name="stage", bufs=2) as stage,
                tc.tile_pool(name="wpool", bufs=1) as wpool,
                tc.tile_pool(name="ppj", bufs=2, space="PSUM") as ppj,
            ):
                def transposed_input(nm, src):
                    """DRAM f32 [s,d] -> SBUF bf16 [d,s] via cast + xbar bounce."""
                    scratch = dram.tile([s, d], BF16, tag=f"sc_{nm}")
                    for i in range(st):
                        t = stage.tile([P, d], BF16, tag="stage_in")
                        nc.gpsimd.dma_start(out=t, in_=src[i * P:(i + 1) * P, :])
                        nc.sync.dma_start(out=scratch[i * P:(i + 1) * P, :], in_=t)
                    xT = xTp.tile([P, nt, s], BF16, tag="xT")
                    for j in range(nt):
                        nc.sync.dma_start_transpose(
                            out=xT[:, j, :], in_=scratch[:, j * P:(j + 1) * P]
                        )
                    return xT

                # keepT = (mask^T == 0)
                kscratch = dram.tile([s, s], BF16, tag="sc_keep")
                for i in range(s // P):
                    mi = stage.tile([P, s], I32, tag="mask_i32")
                    nc.sync.dma_start(out=mi, in_=dmask[i * P:(i + 1) * P, :])
                    mb = stage.tile([P, s], BF16, tag="mask_bf")
                    nc.vector.tensor_scalar(
                        out=mb, in0=mi, scalar1=0, scalar2=None, op0=OP.is_equal
                    )
                    nc.sync.dma_start(out=kscratch[i * P:(i + 1) * P, :], in_=mb)
                for j in range(st):
                    nc.sync.dma_start_transpose(
                        out=keepT[:, j, :], in_=kscratch[:, j * P:(j + 1) * P]
                    )

                # zero-init the DRAM attn_mean accumulator
                ztile = stage.tile([P, s], BF16, tag="zero")
                nc.vector.memset(ztile, 0.0)
                for i in range(st):
                    nc.sync.dma_start(
                        out=meanT_dram[i * P:(i + 1) * P, :], in_=ztile
                    )

                # ---------- phase 0b: projections ----------
                def proj_T(w_dram, x_nm, x_dram, outbuf, bias_row, scale):
                    """outbuf[dout, s] (transposed): lhsT=W[din,dout], rhs=xT."""
                    x_T = transposed_input(x_nm, x_dram)
                    wsb = wpool.tile([P, nt, d], BF16, tag="w")
                    for kt in range(nt):
                        nc.gpsimd.dma_start(
                            out=wsb[:, kt, :], in_=w_dram[kt * P:(kt + 1) * P, :]
                        )
                    for mt in range(nt):
                        ps = ppj.tile([P, s], F32, tag="pj")
                        for n2 in range(nb):
                            cb = slice(n2 * bw, (n2 + 1) * bw)
                            for kt in range(nt):
                                nc.tensor.matmul(
                                    ps[:, cb],
                                    lhsT=wsb[:, kt, mt * P:(mt + 1) * P],
                                    rhs=x_T[:, kt, cb],
                                    start=(kt == 0),
                                    stop=False,
                                )
                            # bias: out[m, n] += bias[m] * 1
                            nc.tensor.matmul(
                                ps[:, cb],
                                lhsT=bias_row[0:1, mt * P:(mt + 1) * P],
                                rhs=ones_row[0:1, 0:bw],
                                start=False,
                                stop=True,
                            )
                        nc.scalar.activation(
                            out=outbuf[:, mt, :], in_=ps, func=AF.Copy, scale=scale
                        )

                proj_T(dWq, "q", dq, QT, brows["bq"], 1.0 / 8.0)
                proj_T(dWk, "k", dk, KT, brows["bk"], 1.0)

                # V natural: out[s, dout]: lhsT = vT[din, s-tile], rhs = Wv
                vT = transposed_input("v", dv)
                wsb = wpool.tile([P, nt, d], BF16, tag="w")
                for kt in range(nt):
                    nc.gpsimd.dma_start(
                        out=wsb[:, kt, :], in_=dWv[kt * P:(kt + 1) * P, :]
                    )
                for mt in range(st):
                    ps = ppj.tile([P, d], F32, tag="pj")
                    for n2 in range(d // bw if d >= bw else 1):
                        cbw = min(bw, d)
                        cb = slice(n2 * cbw, (n2 + 1) * cbw)
                        for kt in range(nt):
                            nc.tensor.matmul(
                                ps[:, cb],
                                lhsT=vT[:, kt, mt * P:(mt + 1) * P],
                                rhs=wsb[:, kt, cb],
                                start=(kt == 0),
                                stop=False,
                            )
                        nc.tensor.matmul(
                            ps[:, cb],
                            lhsT=ones_row[0:1, 0:P],
                            rhs=brows["bv"][0:1, cb],
                            start=False,
                            stop=True,
                        )
                    nc.scalar.activation(
                        out=Vx[:, mt, :, 0:DH],
                        in_=ps.rearrange("p (h dh) -> p h dh", h=h),
                        func=AF.Copy,
                    )
                nc.vector.memset(Vx[:, :, :, DH:DH + 1], 1.0)

            # ---------- heads ----------
            with (
                tc.tile_pool(name="psc", bufs=2, space="PSUM") as psc_pool,
                tc.tile_pool(name="ppv", bufs=1, space="PSUM") as ppv_pool,
                tc.tile_pool(name="prb", bufs=1, space="PSUM") as prb_pool,
                tc.tile_pool(name="hwork", bufs=2) as hwork,
                tc.tile_pool(name="expp", bufs=4) as expp,
                tc.tile_pool(name="expm", bufs=2 * st) as expmp,
                tc.tile_pool(name="attp", bufs=3) as attp,
            ):
                for hh in range(h):
                    ht, ho = hh // hpt, (hh % hpt) * DH
                    ppv = ppv_pool.tile([DH + 1, s], F32, tag="pv")
                    ems = []
                    for kt in range(st):
                        pss = psc_pool.tile([P, s], F32, tag="sc")
                        for n2 in range(nb):
                            cb = slice(n2 * bw, (n2 + 1) * bw)
                            nc.tensor.matmul(
                                pss[:, cb],
                                lhsT=KT[ho:ho + DH, ht, kt * P:(kt + 1) * P],
                                rhs=QT[ho:ho + DH, ht, cb],
                                start=True,
                                stop=True,
                            )
                        et = expp.tile([P, s], BF16, tag="exp")
                        nc.scalar.activation(out=et, in_=pss, func=AF.Exp)
                        em = expmp.tile([P, s], BF16, tag="expm")
                        nc.vector.tensor_tensor(
                            out=em, in0=et, in1=keepT[:, kt, :], op=OP.mult
                        )
                        ems.append(em)
                        for n2 in range(nb):
                            cb = slice(n2 * bw, (n2 + 1) * bw)
                            nc.tensor.matmul(
                                ppv[:, cb],
                                lhsT=Vx[:, kt, hh, :],
                                rhs=em[:, cb],
                                start=(kt == 0),
                                stop=(kt == st - 1),
                            )
                    # softmax denominators: row [1,s] -> [128, st] -> recip
                    # -> transpose back -> broadcast to [128, s]
                    srow = hwork.tile([1, s], F32, tag="srow")
                    nc.scalar.activation(out=srow, in_=ppv[DH:DH + 1, :], func=AF.Copy)
                    p128 = prb_pool.tile([P, st], F32, tag="rb")
                    for j in range(st):
                        nc.tensor.matmul(
                            p128[:, j:j + 1],
                            lhsT=srow[0:1, j * P:(j + 1) * P],
                            rhs=ones_f32[0:1, 0:1],
                            start=True, stop=True,
                        )
                    r128 = hwork.tile([P, st], F32, tag="r128")
                    nc.vector.reciprocal(out=r128, in_=p128)
                    r128b = hwork.tile([P, st], BF16, tag="r128b")
                    nc.vector.tensor_copy(out=r128b, in_=r128)
                    prT = prb_pool.tile([st, P], BF16, tag="rb")
                    nc.tensor.transpose(out=prT, in_=r128b, identity=ident)
                    rT = hwork.tile([st, P], BF16, tag="rT")
                    nc.scalar.activation(out=rT, in_=prT, func=AF.Copy)
                    prb = prb_pool.tile([P, s], F32, tag="rb")
                    for j in range(st):
                        nc.tensor.matmul(
                            prb[:, j * P:(j + 1) * P],
                            lhsT=onehot[:, j, :],
                            rhs=rT,
                            start=True, stop=True,
                        )
                    rb = hwork.tile([P, s], BF16, tag="rb_sb")
                    nc.scalar.activation(out=rb, in_=prb, func=AF.Copy)
                    # context normalize (psum fp32 * rb -> bf16 sbuf)
                    nc.vector.tensor_tensor(
                        out=ctxT[ho:ho + DH, ht, :], in0=ppv[0:DH, :],
                        in1=rb[0:DH, :], op=OP.mult,
                    )
                    # attn_mean accumulation via accumulate-DMA into DRAM
                    for kt in range(st):
                        at = attp.tile([P, s], BF16, tag="attn_tmp")
                        nc.vector.tensor_tensor(
                            out=at, in0=ems[kt], in1=rb, op=OP.mult
                        )
                        nc.gpsimd.dma_start(
                            out=meanT_dram[kt * P:(kt + 1) * P, :], in_=at,
                            accum_op=OP.add,
                        )

            # ---------- epilogue: attn_mean transpose + output proj ----------
            with (
                tc.tile_pool(name="pep", bufs=2, space="PSUM") as pep,
                tc.tile_pool(name="osb", bufs=2) as osb,
                tc.tile_pool(name="wo", bufs=1) as wop,
            ):
                mrows = []
                for kt in range(st):
                    mr = osb.tile([P, s], BF16, tag=f"mrow{kt}")
                    nc.sync.dma_start(out=mr, in_=meanT_dram[kt * P:(kt + 1) * P, :])
                    mrows.append(mr)
                for mt in range(st):
                    psm = pep.tile([P, s], BF16, tag="mt")
                    for kt in range(st):
                        nc.tensor.transpose(
                            out=psm[:, kt * P:(kt + 1) * P],
                            in_=mrows[kt][:, mt * P:(mt + 1) * P],
                            identity=ident,
                        )
                    mo = osb.tile([P, s], F32, tag="mean_out")
                    nc.scalar.activation(
                        out=mo, in_=psm, func=AF.Copy, scale=1.0 / h
                    )
                    nc.sync.dma_start(out=dmean[mt * P:(mt + 1) * P, :], in_=mo)

                wsb = wop.tile([P, nt, d], BF16)
                for kt in range(nt):
                    nc.gpsimd.dma_start(
                        out=wsb[:, kt, :], in_=dWo[kt * P:(kt + 1) * P, :]
                    )
                for mt in range(st):
                    pso = pep.tile([P, d], F32, tag="out")
                    for n2 in range(d // bw if d >= bw else 1):
                        cbw = min(bw, d)
                        cb = slice(n2 * cbw, (n2 + 1) * cbw)
                        for kt in range(nt):
                            nc.tensor.matmul(
                                pso[:, cb],
                                lhsT=ctxT[:, kt, mt * P:(mt + 1) * P],
                                rhs=wsb[:, kt, cb],
                                start=(kt == 0),
                                stop=False,
                            )
                        nc.tensor.matmul(
                            pso[:, cb],
                            lhsT=ones_row[0:1, 0:P],
                            rhs=brows["bo"][0:1, cb],
                            start=False,
                            stop=True,
                        )
                    oo = osb.tile([P, d], F32, tag="out_sb")
                    nc.scalar.activation(out=oo, in_=pso, func=AF.Copy)
                    nc.sync.dma_start(out=dout[mt * P:(mt + 1) * P, :], in_=oo)

    nc.compile()
    return nc


_NC_CACHE = {}


def _get_nc():
    if "nc" not in _NC_CACHE:
        _NC_CACHE["nc"] = build_attention_nc()
    return _NC_CACHE["nc"]


def kernel(k, v, q, attn_mask, Wk, bk, Wv, bv, Wq, bq, Wo, bo, **_ignored):
    from concourse.bass_utils import run_bass_kernel_spmd

    k = np.asarray(k, np.float32)
    v = np.asarray(v, np.float32)
    q = np.asarray(q, np.float32)
    attn_mask = np.asarray(attn_mask, np.int32)
    shared = {
        "Wk": np.asarray(Wk, np.float32), "bk": np.asarray(bk, np.float32),
        "Wv": np.asarray(Wv, np.float32), "bv": np.asarray(bv, np.float32),
        "Wq": np.asarray(Wq, np.float32), "bq": np.asarray(bq, np.float32),
        "Wo": np.asarray(Wo, np.float32), "bo": np.asarray(bo, np.float32),
    }
    in_maps = []
    for b in range(B):
        m = {"q": q[b], "k": k[b], "v": v[b], "attn_mask": attn_mask[b]}
        m.update(shared)
        in_maps.append(m)

    nc = _get_nc()
    res = run_bass_kernel_spmd(nc, in_maps, core_ids=list(range(B)))
    output = np.stack([res.results[b]["output"] for b in range(B)])
    attn_mean = np.stack([res.results[b]["attn_mean"] for b in range(B)])
    return output, attn_mean
